# revision 20
# baseline (speedup 1.0000x reference)
"""NT-Xent / SimCLR contrastive loss on 8 Trainium2 NeuronCores (Bass/Tile).

Problem: zi, zj [4096, 512] f32 -> scalar loss.
  reps = concat(zi, zj)            [8192, 512]
  rn   = reps / max(||reps||, 1e-8)
  sim  = rn @ rn.T                 [8192, 8192]
  pos_i  = sim[i, (i+B) mod 2B]
  denom_i = sum_{j != i} exp(sim_ij / tau)
  loss = mean(-pos/tau + log(denom))

Symmetric decomposition: sim is symmetric, so each unordered block pair is
computed once. Core c owns rows [c*1024, (c+1)*1024) and computes
  - GEMM-A: its rows x column blocks {c, c+1, c+2, c+3} (mod 8), 4096 cols
  - two antipodal quadrants vs block b=(c+4)%8: rows[:512] x Qa and
    rows[512:] x Qb, 512 cols each (the quadrant pairing swaps between core
    c and core b so the four quadrants of the antipodal block pair tile
    exactly).
Row sums of exp come from the activation accumulator; column sums of exp
(the mirrored row-partials destined for other cores' rows) are accumulated
on the vector engine into a [128, 4096] buffer and shipped to the host,
which folds the 128 partitions and assembles denom from all partials
(host also applies log and the final mean, as in the all-reduce of the
sharding hint). Work per core is 0.5625x the full row-block GEMM.

The per-core column order (own block first, then +1,+2,+3, then the
antipodal 1024 with halves swapped on cores >= 4) is arranged by the host,
so the program is SPMD-uniform: the own-block diagonal is always at column
t*128 of group 0 (exact self-exclusion via PSUM extraction + same-LUT exp
cancellation), and the positives fall on the quadrant diagonals (sim is
symmetric, so cores 0-3's quadrant diagonals cover all 8192 positives).

Normalization is on-device (squares via DVE/GpSimd scalar_tensor_tensor,
1/sqrt via Exp(-0.5*Ln) on the single pinned ACT table); normalized rows
roundtrip through DRAM for the DMA-xbar transpose that builds the [K, N]
GEMM operand.
"""

import sys

for _p in ("/opt/trn_rl_repo",):
    if _p not in sys.path:
        sys.path.insert(0, _p)

from contextlib import ExitStack

import ml_dtypes
import numpy as np

TAU = 0.07
B, D = 4096, 512
NCORES = 8
ROWS = 2 * B              # 8192
RPC = ROWS // NCORES      # 1024 rows per core
NM = RPC // 128           # 8 m-tiles per core
KC = D // 128             # 4 contraction chunks
CTOT = 5 * RPC            # 5120 columns per core (4 blocks + antipodal)
NCT = CTOT // 128         # 40 natural col tiles
NSEG = CTOT // 1024       # 5 transpose segments of 1024 rows
NA = 4 * RPC              # 4096 GEMM-A columns
NGA = NA // 1024          # 4 A-groups of 1024 cols
CJ = 4096                 # colacc width (cols 1024..5120)

_prog_cache = {}


def _build_program():
    import concourse.bacc as bacc
    import concourse.tile as tile
    import concourse.mybir as mybir

    dt = mybir.dt
    Alu = mybir.AluOpType
    Act = mybir.ActivationFunctionType

    nc = bacc.Bacc("TRN2", target_bir_lowering=False, debug=False,
                   enable_asserts=False, num_devices=NCORES)

    cols_nat = nc.dram_tensor("cols_nat", [NCT, 128, D], dt.bfloat16,
                              kind="ExternalInput").ap()
    ident_f32 = nc.dram_tensor("ident_f32", [128, 128], dt.float32,
                               kind="ExternalInput").ap()
    out = nc.dram_tensor("out", [128, 16], dt.float32,
                         kind="ExternalOutput").ap()
    cacc_out = nc.dram_tensor("cacc_out", [128, CJ], dt.bfloat16,
                              kind="ExternalOutput").ap()

    inv_tau = float(1.0 / TAU)

    with tile.TileContext(nc) as tc, ExitStack() as ctx:
        const = ctx.enter_context(tc.tile_pool(name="const", bufs=1))
        persist = ctx.enter_context(tc.tile_pool(name="persist", bufs=1))
        dramp = ctx.enter_context(tc.tile_pool(name="dramp", bufs=1,
                                               space="DRAM"))
        xin = ctx.enter_context(tc.tile_pool(name="xin", bufs=NSEG))
        rnp = ctx.enter_context(tc.tile_pool(name="rnp", bufs=NSEG))
        scrp = ctx.enter_context(tc.tile_pool(name="scrp", bufs=3))
        normp = ctx.enter_context(tc.tile_pool(name="normp", bufs=NSEG))
        ep = ctx.enter_context(tc.tile_pool(name="ep", bufs=4))
        smallp = ctx.enter_context(tc.tile_pool(name="smallp", bufs=4))
        psA = ctx.enter_context(tc.tile_pool(name="psA", bufs=3,
                                             space="PSUM"))
        psB = ctx.enter_context(tc.tile_pool(name="psB", bufs=2,
                                             space="PSUM"))

        i32 = const.tile([128, 128], dt.float32, tag="i32")
        nc.sync.dma_start(i32[:], ident_f32[:])

        # resident transposed GEMM operand: chunk c at [:, c, :]
        rnT = persist.tile([128, KC * CTOT], dt.bfloat16, tag="rnT")
        rnT_v = rnT[:].rearrange("p (c w) -> p c w", c=KC)
        colacc = persist.tile([128, CJ], dt.bfloat16, tag="colacc")
        rs = persist.tile([128, 5 * NM], dt.float32, tag="rs")
        dv = persist.tile([128, NM], dt.float32, tag="dv")
        outbuf = persist.tile([128, 16], dt.float32, tag="outbuf")

        # DRAM scratch for normalized rows (transpose-DMA source)
        segs = [dramp.tile([8, 128, D], dt.bfloat16, tag=f"seg{s}",
                           name=f"seg{s}") for s in range(NSEG)]

        # ---- normalize + transpose pipeline (1024-row segments) ----
        # DMA dispatch is spread across the two HWDGE queues (sync/SP and
        # scalar/Act) plus GpSimd SWDGE for stores: a single queue pays
        # ~0.7us dispatch per op and serializes the whole pipeline.
        # Tile's hazard tracker attaches a matmul's rnT dependency only to
        # the Ldweights (stationary read) - the MOVING operand read has no
        # edge to the DMA transpose that writes it. Record transpose insts
        # and add the edges manually (PE is in-order, so the first matmul
        # touching a (segment, chunk) orders all later ones).
        import bass_rust
        tr_inst = {}

        def mm_dep(inst, s, c):
            inst.ins.add_dependency(tr_inst[(s, c)].ins.name,
                                    bass_rust.DependencyInfo.SYNC_ONLY)

        # prefetch all segment inputs first: the sync queue is in-order, so
        # loads must not sit behind stores/transposes that wait on compute
        xs = []
        for s in range(NSEG):
            x8 = xin.tile([128, 8 * D], dt.bfloat16, tag="x8")
            nc.sync.dma_start(
                x8[:, :4 * D].rearrange("p (a d) -> p a d", a=4),
                cols_nat[8 * s:8 * s + 4].rearrange("a p d -> p a d"))
            nc.sync.dma_start(
                x8[:, 4 * D:].rearrange("p (a d) -> p a d", a=4),
                cols_nat[8 * s + 4:8 * s + 8].rearrange("a p d -> p a d"))
            xs.append(x8)
        for s in range(NSEG):
            # store + transposes of a segment share one in-order queue
            # (alternating per segment): cross-queue write->read ordering on
            # DRAM scratch proved racy (Pool SWDGE stores).
            q = nc.sync if s % 2 == 0 else nc.scalar
            x8 = xs[s]
            n2 = normp.tile([128, 8], dt.float32, tag="n2")
            for k in range(8):
                scr = scrp.tile([128, D], dt.bfloat16, tag="scr512")
                nc.vector.scalar_tensor_tensor(
                    out=scr[:], in0=x8[:, k * D:(k + 1) * D], scalar=1.0,
                    in1=x8[:, k * D:(k + 1) * D], op0=Alu.mult, op1=Alu.mult,
                    accum_out=n2[:, k:k + 1])
            nc.vector.tensor_scalar(out=n2[:], in0=n2[:], scalar1=1e-16,
                                    scalar2=None, op0=Alu.max)
            # inv = n2 ** -0.5 via Exp(-0.5 * Ln(n2)): both functions live in
            # the single pinned ACT table.
            lng = normp.tile([128, 8], dt.float32, tag="lng")
            nc.scalar.activation(lng[:], n2[:], Act.Ln)
            inv = normp.tile([128, 8], dt.float32, tag="inv")
            nc.scalar.activation(inv[:], lng[:], Act.Exp, scale=-0.5)
            rn8 = rnp.tile([128, 8 * D], dt.bfloat16, tag="rn8")
            for k in range(8):
                nc.vector.tensor_scalar_mul(rn8[:, k * D:(k + 1) * D],
                                            x8[:, k * D:(k + 1) * D],
                                            inv[:, k:k + 1])
            q.dma_start(segs[s][:].rearrange("a p d -> p a d"),
                        rn8[:].rearrange("p (a d) -> p a d", a=8))
            s2d = segs[s][:].rearrange("a p d -> (a p) d")
            for c in range(KC):
                tr_inst[(s, c)] = q.dma_start_transpose(
                    rnT_v[:, c, s * 1024:(s + 1) * 1024],
                    s2d[:, c * 128:(c + 1) * 128])

        # ---- GEMM-A: 4 groups of 1024 columns, 8 m-tiles each ----
        for g in range(NGA):
            for t in range(NM):
                ps = psA.tile([128, 1024], dt.float32, tag="ps")
                for c in range(KC):
                    for h in range(2):
                        mm = nc.tensor.matmul(
                            ps[:, h * 512:(h + 1) * 512],
                            rnT_v[:, c, t * 128:(t + 1) * 128],
                            rnT_v[:, c, g * 1024 + h * 512:
                                  g * 1024 + (h + 1) * 512],
                            start=(c == 0), stop=(c == KC - 1))
                        if t == 0 and h == 0:
                            mm_dep(mm, g, c)  # moving operand = segment g
                if g == 0:
                    # exact self-sim extraction (diag of own block at t*128)
                    scr = scrp.tile([128, 128], dt.float32, tag="scrd")
                    nc.vector.scalar_tensor_tensor(
                        out=scr[:], in0=ps[:, t * 128:(t + 1) * 128],
                        scalar=1.0, in1=i32[:], op0=Alu.mult, op1=Alu.mult,
                        accum_out=dv[:, t:t + 1])
                    e0 = ep.tile([128, 1024], dt.float32, tag="e0")
                    nc.scalar.activation(e0[:], ps[:], Act.Exp,
                                         scale=inv_tau,
                                         accum_out=rs[:, t * 5:t * 5 + 1])
                else:
                    e = ep.tile([128, 1024], dt.bfloat16, tag="e")
                    nc.scalar.activation(e[:], ps[:], Act.Exp,
                                         scale=inv_tau,
                                         accum_out=rs[:, t * 5 + g:
                                                      t * 5 + g + 1])
                    creg = slice((g - 1) * 1024, g * 1024)
                    if t == 0:
                        nc.vector.tensor_scalar(out=colacc[:, creg],
                                                in0=e[:], scalar1=0.0,
                                                scalar2=None, op0=Alu.add)
                    else:
                        nc.vector.tensor_add(colacc[:, creg],
                                             colacc[:, creg], e[:])
            # colacc region (g-1) final after t==NM-1: stream it out early
            if g > 0:
                nc.sync.dma_start(cacc_out[:, (g - 1) * 1024:g * 1024],
                                  colacc[:, (g - 1) * 1024:g * 1024])

        # ---- antipodal quadrants: 512 cols per m-tile ----
        for t in range(NM):
            qcol = NA + (0 if t < 4 else 512)          # rnT col offset
            dcol = t * 128 - (0 if t < 4 else 512)     # diag pos in quadrant
            ps = psB.tile([128, 512], dt.float32, tag="psq")
            for c in range(KC):
                mm = nc.tensor.matmul(
                    ps[:], rnT_v[:, c, t * 128:(t + 1) * 128],
                    rnT_v[:, c, qcol:qcol + 512],
                    start=(c == 0), stop=(c == KC - 1))
                if t == 0:
                    mm_dep(mm, NSEG - 1, c)  # quadrant cols = last segment
            # positives: quadrant diagonal (pre-exp, f32)
            scr = scrp.tile([128, 128], dt.float32, tag="scrd")
            nc.vector.scalar_tensor_tensor(
                out=scr[:], in0=ps[:, dcol:dcol + 128], scalar=1.0,
                in1=i32[:], op0=Alu.mult, op1=Alu.mult,
                accum_out=outbuf[:, 8 + t:9 + t])
            eq = ep.tile([128, 512], dt.bfloat16, tag="eq")
            nc.scalar.activation(eq[:], ps[:], Act.Exp, scale=inv_tau,
                                 accum_out=rs[:, t * 5 + 4:t * 5 + 5])
            creg = slice(3072 + (0 if t < 4 else 512),
                         3584 + (0 if t < 4 else 512))
            if t % 4 == 0:
                nc.vector.tensor_scalar(out=colacc[:, creg], in0=eq[:],
                                        scalar1=0.0, scalar2=None,
                                        op0=Alu.add)
            else:
                nc.vector.tensor_add(colacc[:, creg], colacc[:, creg], eq[:])
        nc.sync.dma_start(cacc_out[:, 3072:4096], colacc[:, 3072:4096])

        # ---- epilogue ----
        selfexp = smallp.tile([128, NM], dt.float32, tag="selfexp")
        nc.scalar.activation(selfexp[:], dv[:], Act.Exp, scale=inv_tau)
        rsum = smallp.tile([128, NM], dt.float32, tag="rsum")
        for t in range(NM):
            nc.vector.reduce_sum(rsum[:, t:t + 1], rs[:, t * 5:(t + 1) * 5],
                                 axis=mybir.AxisListType.X)
        nc.vector.tensor_sub(outbuf[:, 0:8], rsum[:], selfexp[:])
        nc.sync.dma_start(out[:], outbuf[:])

    # Pin bacc's activation-table choice to the one table holding Ln+Exp+Copy
    # so exactly one ACT table load is emitted.
    import concourse.bacc as bacc_mod
    _orig_tables = bacc_mod.get_activation_tables

    def _only_lnexp(arch):
        keep = "natural_log_exp_and_others"
        return {k: (v if k == keep else set())
                for k, v in _orig_tables(arch).items()}

    bacc_mod.get_activation_tables = _only_lnexp
    try:
        nc.compile()
    finally:
        bacc_mod.get_activation_tables = _orig_tables
    return nc


def _col_rows(c):
    """Global row indices of core c's 5120 GEMM columns, in rnT order."""
    b = (c + 4) % NCORES
    idxs = [np.arange(((c + d) % NCORES) * RPC, ((c + d) % NCORES + 1) * RPC)
            for d in range(4)]
    if c < 4:
        q = np.arange(b * RPC, (b + 1) * RPC)
    else:
        q = np.concatenate([np.arange(b * RPC + 512, (b + 1) * RPC),
                            np.arange(b * RPC, b * RPC + 512)])
    idxs.append(q)
    return np.concatenate(idxs)


def _host_inputs(zi, zj):
    reps = np.concatenate([np.asarray(zi, np.float32),
                           np.asarray(zj, np.float32)], axis=0)
    reps_bf = reps.astype(ml_dtypes.bfloat16)
    ident_f32 = np.eye(128, dtype=np.float32)
    in_maps = []
    for c in range(NCORES):
        cols = np.ascontiguousarray(
            reps_bf[_col_rows(c)].reshape(NCT, 128, D))
        in_maps.append({"cols_nat": cols, "ident_f32": ident_f32})
    return in_maps


def _postprocess(results):
    denom = np.zeros(ROWS, np.float64)
    pos = np.zeros(ROWS, np.float64)
    for c in range(NCORES):
        o = np.asarray(results[c]["out"], np.float64)        # [128, 16]
        ca = np.asarray(results[c]["cacc_out"], np.float64)  # [128, 4096]
        cr = _col_rows(c)
        for t in range(NM):
            rows = slice(c * RPC + t * 128, c * RPC + (t + 1) * 128)
            denom[rows] += o[:, t]
        # colsum partials: fold partitions, scatter to owning rows
        colsum = ca.sum(axis=0)                              # [4096]
        np.add.at(denom, cr[1024:], colsum)
        if c < 4:
            opos = o[:, 8:16].T.reshape(-1)                  # [1024]
            rows = np.arange(c * RPC, (c + 1) * RPC)
            pos[rows] = opos
            pos[cr[4096:]] = opos
    loss = np.mean(-pos / TAU + np.log(denom))
    return np.asarray(loss, dtype=np.float32)


def kernel(zi, zj, _trace=False):
    from concourse.bass_utils import run_bass_kernel_spmd

    if "nc" not in _prog_cache:
        _prog_cache["nc"] = _build_program()
    nc = _prog_cache["nc"]
    in_maps = _host_inputs(zi, zj)
    res = run_bass_kernel_spmd(nc, in_maps, list(range(NCORES)),
                               trace=_trace)
    _prog_cache["last_result"] = res
    return _postprocess(res.results)


# revision 24
# speedup vs baseline: 1.0059x; 1.0059x over previous
"""NT-Xent / SimCLR contrastive loss on 8 Trainium2 NeuronCores (Bass/Tile).

Problem: zi, zj [4096, 512] f32 -> scalar loss.
  reps = concat(zi, zj)            [8192, 512]
  rn   = reps / max(||reps||, 1e-8)
  sim  = rn @ rn.T                 [8192, 8192]
  pos_i  = sim[i, (i+B) mod 2B]
  denom_i = sum_{j != i} exp(sim_ij / tau)
  loss = mean(-pos/tau + log(denom))

Symmetric decomposition: sim is symmetric, so each unordered block pair is
computed once. Core c owns rows [c*1024, (c+1)*1024) and computes
  - GEMM-A: its rows x column blocks {c, c+1, c+2, c+3} (mod 8), 4096 cols
  - two antipodal quadrants vs block b=(c+4)%8: rows[:512] x Qa and
    rows[512:] x Qb, 512 cols each (the quadrant pairing swaps between core
    c and core b so the four quadrants of the antipodal block pair tile
    exactly).
Row sums of exp come from the activation accumulator; column sums of exp
(the mirrored row-partials destined for other cores' rows) are accumulated
on the vector engine into a [128, 4096] buffer and shipped to the host,
which folds the 128 partitions and assembles denom from all partials
(host also applies log and the final mean, as in the all-reduce of the
sharding hint). Work per core is 0.5625x the full row-block GEMM.

The per-core column order (own block first, then +1,+2,+3, then the
antipodal 1024 with halves swapped on cores >= 4) is arranged by the host,
so the program is SPMD-uniform: the own-block diagonal is always at column
t*128 of group 0 (exact self-exclusion via PSUM extraction + same-LUT exp
cancellation), and the positives fall on the quadrant diagonals (sim is
symmetric, so cores 0-3's quadrant diagonals cover all 8192 positives).

Normalization is on-device (squares via DVE/GpSimd scalar_tensor_tensor,
1/sqrt via Exp(-0.5*Ln) on the single pinned ACT table); normalized rows
roundtrip through DRAM for the DMA-xbar transpose that builds the [K, N]
GEMM operand.
"""

import sys

for _p in ("/opt/trn_rl_repo",):
    if _p not in sys.path:
        sys.path.insert(0, _p)

from contextlib import ExitStack

import ml_dtypes
import numpy as np

TAU = 0.07
B, D = 4096, 512
NCORES = 8
ROWS = 2 * B              # 8192
RPC = ROWS // NCORES      # 1024 rows per core
NM = RPC // 128           # 8 m-tiles per core
KC = D // 128             # 4 contraction chunks
CTOT = 5 * RPC            # 5120 columns per core (4 blocks + antipodal)
NCT = CTOT // 128         # 40 natural col tiles
NSEG = CTOT // 1024       # 5 transpose segments of 1024 rows
NA = 4 * RPC              # 4096 GEMM-A columns
NGA = NA // 1024          # 4 A-groups of 1024 cols
CJ = 4096                 # colacc width (cols 1024..5120)

_prog_cache = {}


def _build_program():
    import concourse.bacc as bacc
    import concourse.tile as tile
    import concourse.mybir as mybir

    dt = mybir.dt
    Alu = mybir.AluOpType
    Act = mybir.ActivationFunctionType

    nc = bacc.Bacc("TRN2", target_bir_lowering=False, debug=False,
                   enable_asserts=False, num_devices=NCORES)

    cols_nat = nc.dram_tensor("cols_nat", [NCT, 128, D], dt.bfloat16,
                              kind="ExternalInput").ap()
    ident_f32 = nc.dram_tensor("ident_f32", [128, 128], dt.float32,
                               kind="ExternalInput").ap()
    out = nc.dram_tensor("out", [128, 16], dt.float32,
                         kind="ExternalOutput").ap()
    cacc_out = nc.dram_tensor("cacc_out", [128, CJ], dt.bfloat16,
                              kind="ExternalOutput").ap()

    inv_tau = float(1.0 / TAU)

    with tile.TileContext(nc) as tc, ExitStack() as ctx:
        const = ctx.enter_context(tc.tile_pool(name="const", bufs=1))
        persist = ctx.enter_context(tc.tile_pool(name="persist", bufs=1))
        dramp = ctx.enter_context(tc.tile_pool(name="dramp", bufs=1,
                                               space="DRAM"))
        xin = ctx.enter_context(tc.tile_pool(name="xin", bufs=NSEG))
        rnp = ctx.enter_context(tc.tile_pool(name="rnp", bufs=NSEG))
        scrp = ctx.enter_context(tc.tile_pool(name="scrp", bufs=3))
        normp = ctx.enter_context(tc.tile_pool(name="normp", bufs=NSEG))
        ep = ctx.enter_context(tc.tile_pool(name="ep", bufs=4))
        smallp = ctx.enter_context(tc.tile_pool(name="smallp", bufs=4))
        psA = ctx.enter_context(tc.tile_pool(name="psA", bufs=3,
                                             space="PSUM"))
        psB = ctx.enter_context(tc.tile_pool(name="psB", bufs=2,
                                             space="PSUM"))

        i32 = const.tile([128, 128], dt.float32, tag="i32")
        nc.sync.dma_start(i32[:], ident_f32[:])

        # resident transposed GEMM operand: chunk c at [:, c, :]
        rnT = persist.tile([128, KC * CTOT], dt.bfloat16, tag="rnT")
        rnT_v = rnT[:].rearrange("p (c w) -> p c w", c=KC)
        colacc = persist.tile([128, CJ], dt.bfloat16, tag="colacc")
        rs = persist.tile([128, 5 * NM], dt.float32, tag="rs")
        dv = persist.tile([128, NM], dt.float32, tag="dv")
        outbuf = persist.tile([128, 16], dt.float32, tag="outbuf")

        # DRAM scratch for normalized rows (transpose-DMA source)
        segs = [dramp.tile([8, 128, D], dt.bfloat16, tag=f"seg{s}",
                           name=f"seg{s}") for s in range(NSEG)]

        # ---- normalize + transpose pipeline (1024-row segments) ----
        # DMA dispatch is spread across the two HWDGE queues (sync/SP and
        # scalar/Act) plus GpSimd SWDGE for stores: a single queue pays
        # ~0.7us dispatch per op and serializes the whole pipeline.
        # Tile's hazard tracker attaches a matmul's rnT dependency only to
        # the Ldweights (stationary read) - the MOVING operand read has no
        # edge to the DMA transpose that writes it. Record transpose insts
        # and add the edges manually (PE is in-order, so the first matmul
        # touching a (segment, chunk) orders all later ones).
        import bass_rust
        tr_inst = {}

        def mm_dep(inst, s, c):
            inst.ins.add_dependency(tr_inst[(s, c)].ins.name,
                                    bass_rust.DependencyInfo.SYNC_ONLY)

        # PSUM WAR edges are also missing: a slot-recycling matmul
        # (start=True resets the bank) must wait for the previous
        # occupant's readers (exp / diag STT). Track readers per pool slot.
        psA_readers = {}   # slot -> [inst names]
        psB_readers = {}

        def war_dep(mm, readers):
            for rn_ in readers:
                mm.ins.add_dependency(rn_, bass_rust.DependencyInfo.SYNC_ONLY)

        # prefetch all segment inputs first: the sync queue is in-order, so
        # loads must not sit behind stores/transposes that wait on compute
        xs = []
        for s in range(NSEG):
            x8 = xin.tile([128, 8 * D], dt.bfloat16, tag="x8")
            nc.sync.dma_start(
                x8[:, :4 * D].rearrange("p (a d) -> p a d", a=4),
                cols_nat[8 * s:8 * s + 4].rearrange("a p d -> p a d"))
            nc.sync.dma_start(
                x8[:, 4 * D:].rearrange("p (a d) -> p a d", a=4),
                cols_nat[8 * s + 4:8 * s + 8].rearrange("a p d -> p a d"))
            xs.append(x8)
        for s in range(NSEG):
            # store + transposes of a segment share one in-order queue
            # (alternating per segment): cross-queue write->read ordering on
            # DRAM scratch proved racy (Pool SWDGE stores).
            q = nc.sync if s % 2 == 0 else nc.scalar
            x8 = xs[s]
            n2 = normp.tile([128, 8], dt.float32, tag="n2")
            for k in range(8):
                scr = scrp.tile([128, D], dt.bfloat16, tag="scr512")
                nc.vector.scalar_tensor_tensor(
                    out=scr[:], in0=x8[:, k * D:(k + 1) * D], scalar=1.0,
                    in1=x8[:, k * D:(k + 1) * D], op0=Alu.mult, op1=Alu.mult,
                    accum_out=n2[:, k:k + 1])
            nc.vector.tensor_scalar(out=n2[:], in0=n2[:], scalar1=1e-16,
                                    scalar2=None, op0=Alu.max)
            # inv = n2 ** -0.5 via Exp(-0.5 * Ln(n2)): both functions live in
            # the single pinned ACT table.
            lng = normp.tile([128, 8], dt.float32, tag="lng")
            nc.scalar.activation(lng[:], n2[:], Act.Ln)
            inv = normp.tile([128, 8], dt.float32, tag="inv")
            nc.scalar.activation(inv[:], lng[:], Act.Exp, scale=-0.5)
            rn8 = rnp.tile([128, 8 * D], dt.bfloat16, tag="rn8")
            for k in range(8):
                nc.vector.tensor_scalar_mul(rn8[:, k * D:(k + 1) * D],
                                            x8[:, k * D:(k + 1) * D],
                                            inv[:, k:k + 1])
            q.dma_start(segs[s][:].rearrange("a p d -> p a d"),
                        rn8[:].rearrange("p (a d) -> p a d", a=8))
            s2d = segs[s][:].rearrange("a p d -> (a p) d")
            for c in range(KC):
                tr_inst[(s, c)] = q.dma_start_transpose(
                    rnT_v[:, c, s * 1024:(s + 1) * 1024],
                    s2d[:, c * 128:(c + 1) * 128])

        # ---- GEMM-A: 4 groups of 1024 columns, 8 m-tiles each ----
        for g in range(NGA):
            for t in range(NM):
                slot = (g * NM + t) % 3
                ps = psA.tile([128, 1024], dt.float32, tag="ps")
                for c in range(KC):
                    for h in range(2):
                        mm = nc.tensor.matmul(
                            ps[:, h * 512:(h + 1) * 512],
                            rnT_v[:, c, t * 128:(t + 1) * 128],
                            rnT_v[:, c, g * 1024 + h * 512:
                                  g * 1024 + (h + 1) * 512],
                            start=(c == 0), stop=(c == KC - 1))
                        if t == 0 and h == 0:
                            mm_dep(mm, g, c)  # moving operand = segment g
                        if c == 0:
                            war_dep(mm, psA_readers.get(slot, ()))
                readers = []
                if g == 0:
                    # exact self-sim extraction (diag of own block at t*128)
                    scr = scrp.tile([128, 128], dt.float32, tag="scrd")
                    stt = nc.vector.scalar_tensor_tensor(
                        out=scr[:], in0=ps[:, t * 128:(t + 1) * 128],
                        scalar=1.0, in1=i32[:], op0=Alu.mult, op1=Alu.mult,
                        accum_out=dv[:, t:t + 1])
                    readers.append(stt.ins.name)
                    e0 = ep.tile([128, 1024], dt.float32, tag="e0")
                    ex = nc.scalar.activation(e0[:], ps[:], Act.Exp,
                                              scale=inv_tau,
                                              accum_out=rs[:, t * 5:
                                                           t * 5 + 1])
                    readers.append(ex.ins.name)
                else:
                    e = ep.tile([128, 1024], dt.bfloat16, tag="e")
                    ex = nc.scalar.activation(e[:], ps[:], Act.Exp,
                                              scale=inv_tau,
                                              accum_out=rs[:, t * 5 + g:
                                                           t * 5 + g + 1])
                    readers.append(ex.ins.name)
                    creg = slice((g - 1) * 1024, g * 1024)
                    if t == 0:
                        nc.vector.tensor_scalar(out=colacc[:, creg],
                                                in0=e[:], scalar1=0.0,
                                                scalar2=None, op0=Alu.add)
                    else:
                        nc.vector.tensor_add(colacc[:, creg],
                                             colacc[:, creg], e[:])
                psA_readers[slot] = readers
            # colacc region (g-1) final after t==NM-1: stream it out early
            if g > 0:
                nc.sync.dma_start(cacc_out[:, (g - 1) * 1024:g * 1024],
                                  colacc[:, (g - 1) * 1024:g * 1024])

        # ---- antipodal quadrants: 512 cols per m-tile ----
        for t in range(NM):
            qcol = NA + (0 if t < 4 else 512)          # rnT col offset
            dcol = t * 128 - (0 if t < 4 else 512)     # diag pos in quadrant
            ps = psB.tile([128, 512], dt.float32, tag="psq")
            for c in range(KC):
                mm = nc.tensor.matmul(
                    ps[:], rnT_v[:, c, t * 128:(t + 1) * 128],
                    rnT_v[:, c, qcol:qcol + 512],
                    start=(c == 0), stop=(c == KC - 1))
                if t == 0:
                    mm_dep(mm, NSEG - 1, c)  # quadrant cols = last segment
                if c == 0:
                    war_dep(mm, psB_readers.get(t % 2, ()))
            # positives: quadrant diagonal (pre-exp, f32)
            scr = scrp.tile([128, 128], dt.float32, tag="scrd")
            stt = nc.vector.scalar_tensor_tensor(
                out=scr[:], in0=ps[:, dcol:dcol + 128], scalar=1.0,
                in1=i32[:], op0=Alu.mult, op1=Alu.mult,
                accum_out=outbuf[:, 8 + t:9 + t])
            eq = ep.tile([128, 512], dt.bfloat16, tag="eq")
            ex = nc.scalar.activation(eq[:], ps[:], Act.Exp, scale=inv_tau,
                                      accum_out=rs[:, t * 5 + 4:t * 5 + 5])
            psB_readers[t % 2] = [stt.ins.name, ex.ins.name]
            creg = slice(3072 + (0 if t < 4 else 512),
                         3584 + (0 if t < 4 else 512))
            if t % 4 == 0:
                nc.vector.tensor_scalar(out=colacc[:, creg], in0=eq[:],
                                        scalar1=0.0, scalar2=None,
                                        op0=Alu.add)
            else:
                nc.vector.tensor_add(colacc[:, creg], colacc[:, creg], eq[:])
        nc.sync.dma_start(cacc_out[:, 3072:4096], colacc[:, 3072:4096])

        # ---- epilogue ----
        selfexp = smallp.tile([128, NM], dt.float32, tag="selfexp")
        nc.scalar.activation(selfexp[:], dv[:], Act.Exp, scale=inv_tau)
        rsum = smallp.tile([128, NM], dt.float32, tag="rsum")
        for t in range(NM):
            nc.vector.reduce_sum(rsum[:, t:t + 1], rs[:, t * 5:(t + 1) * 5],
                                 axis=mybir.AxisListType.X)
        nc.vector.tensor_sub(outbuf[:, 0:8], rsum[:], selfexp[:])
        nc.sync.dma_start(out[:], outbuf[:])

    # Pin bacc's activation-table choice to the one table holding Ln+Exp+Copy
    # so exactly one ACT table load is emitted.
    import concourse.bacc as bacc_mod
    _orig_tables = bacc_mod.get_activation_tables

    def _only_lnexp(arch):
        keep = "natural_log_exp_and_others"
        return {k: (v if k == keep else set())
                for k, v in _orig_tables(arch).items()}

    bacc_mod.get_activation_tables = _only_lnexp
    try:
        nc.compile()
    finally:
        bacc_mod.get_activation_tables = _orig_tables
    return nc


def _col_rows(c):
    """Global row indices of core c's 5120 GEMM columns, in rnT order."""
    b = (c + 4) % NCORES
    idxs = [np.arange(((c + d) % NCORES) * RPC, ((c + d) % NCORES + 1) * RPC)
            for d in range(4)]
    if c < 4:
        q = np.arange(b * RPC, (b + 1) * RPC)
    else:
        q = np.concatenate([np.arange(b * RPC + 512, (b + 1) * RPC),
                            np.arange(b * RPC, b * RPC + 512)])
    idxs.append(q)
    return np.concatenate(idxs)


def _host_inputs(zi, zj):
    reps = np.concatenate([np.asarray(zi, np.float32),
                           np.asarray(zj, np.float32)], axis=0)
    reps_bf = reps.astype(ml_dtypes.bfloat16)
    ident_f32 = np.eye(128, dtype=np.float32)
    in_maps = []
    for c in range(NCORES):
        cols = np.ascontiguousarray(
            reps_bf[_col_rows(c)].reshape(NCT, 128, D))
        in_maps.append({"cols_nat": cols, "ident_f32": ident_f32})
    return in_maps


def _postprocess(results):
    denom = np.zeros(ROWS, np.float64)
    pos = np.zeros(ROWS, np.float64)
    for c in range(NCORES):
        o = np.asarray(results[c]["out"], np.float64)        # [128, 16]
        ca = np.asarray(results[c]["cacc_out"], np.float64)  # [128, 4096]
        cr = _col_rows(c)
        for t in range(NM):
            rows = slice(c * RPC + t * 128, c * RPC + (t + 1) * 128)
            denom[rows] += o[:, t]
        # colsum partials: fold partitions, scatter to owning rows
        colsum = ca.sum(axis=0)                              # [4096]
        np.add.at(denom, cr[1024:], colsum)
        if c < 4:
            opos = o[:, 8:16].T.reshape(-1)                  # [1024]
            rows = np.arange(c * RPC, (c + 1) * RPC)
            pos[rows] = opos
            pos[cr[4096:]] = opos
    loss = np.mean(-pos / TAU + np.log(denom))
    return np.asarray(loss, dtype=np.float32)


def kernel(zi, zj, _trace=False):
    from concourse.bass_utils import run_bass_kernel_spmd

    if "nc" not in _prog_cache:
        _prog_cache["nc"] = _build_program()
    nc = _prog_cache["nc"]
    in_maps = _host_inputs(zi, zj)
    res = run_bass_kernel_spmd(nc, in_maps, list(range(NCORES)),
                               trace=_trace)
    _prog_cache["last_result"] = res
    return _postprocess(res.results)
